# revision 1
# baseline (speedup 1.0000x reference)
"""CIN (xDeepFM CompressedInteractionNetwork) forward on 8 TRN2 NeuronCores.

Strategy (pure data parallelism, hardcoded from the problem spec):
  - batch 4096 -> 512 per core; each core processes 64 "tiles" of 8 batch
    elements; matmul free dim = 512 columns = (8 batch x 64 embed).
  - layer l: out[o, col] = relu( sum_c W[o,c] * z[c, col] + b[o] ) where
    z[f*Hin+j, col] = x0[f, col] * h[j, col].  z is materialized on the
    vector engine as bf16 tensor_tensor multiplies:
      in0 = XR (x0 rows broadcast across all 128 partitions; DMA'd from HBM
            with a stride-0 partition dim), in1 = h tile broadcast along a
            stride-0 free dim.
  - matmuls in bf16 (full PE rate; fp32 would be 4x slower), fp32 PSUM.
  - ScalarE applies bias+relu straight out of PSUM (per-partition bias),
    emitting bf16 h-halves and fp32 pooled-halves.
  - pooling (sum over embed dim) via vector tensor_reduce into per-chunk
    accumulators; final FC = 4 fp32 matmuls at the end; fc_b added on host.

bf16 end-to-end error vs fp32 reference measured at ~7e-4 L2 relative.
"""

import sys

sys.path.insert(0, "/opt/trn_rl_repo")

import numpy as np
import ml_dtypes
from contextlib import ExitStack

N_CORES = 8
B = 4096
F = 32
E = 64
BC = B // N_CORES  # 512 batch elements per core
NB = 8             # batch elements per tile
COLS = NB * E      # 512 matmul columns per tile
NT = BC // NB      # 64 tiles per core
O = 256            # conv out channels per layer
H = 128            # h channels (split_half) for layers 1,2

_CACHE = {}


def _build(n_tiles=NT, debug=False):
    import concourse.bass as bass  # noqa: F401
    import concourse.mybir as mybir
    import concourse.tile as tile
    from concourse import bacc

    dt = mybir.dt
    AF = mybir.ActivationFunctionType
    ALU = mybir.AluOpType
    AX = mybir.AxisListType

    nc = bacc.Bacc("TRN2", target_bir_lowering=False, debug=False,
                   num_devices=N_CORES)

    xb = nc.declare_dram_parameter("xb", [F, BC, E], dt.bfloat16, isOutput=False)
    w0t = nc.declare_dram_parameter("w0t", [F * F, O], dt.bfloat16, isOutput=False)
    w1t = nc.declare_dram_parameter("w1t", [F * H, O], dt.bfloat16, isOutput=False)
    w2t = nc.declare_dram_parameter("w2t", [F * H, O], dt.bfloat16, isOutput=False)
    b0 = nc.declare_dram_parameter("b0", [O], dt.float32, isOutput=False)
    b1 = nc.declare_dram_parameter("b1", [O], dt.float32, isOutput=False)
    b2 = nc.declare_dram_parameter("b2", [O], dt.float32, isOutput=False)
    pout = nc.declare_dram_parameter("pout", [4, 128, n_tiles * NB],
                                     dt.float32, isOutput=True)
    dbg = {}
    if debug:
        for nm in ["dP0", "dP1", "dP2a", "dP2b"]:
            dbg[nm] = nc.declare_dram_parameter(nm, [128, n_tiles * NB],
                                                dt.float32, isOutput=True)
        for nm in ["dz0", "dh1", "dh2"]:
            dbg[nm] = nc.declare_dram_parameter(nm, [128, 8 * COLS] if nm == "dz0"
                                                else [128, COLS],
                                                dt.float32, isOutput=True)

    with ExitStack() as ctx:
        tc = ctx.enter_context(tile.TileContext(nc))
        const = ctx.enter_context(tc.tile_pool(name="const", bufs=1))

        # ---- persistent weights / biases ----
        lw0 = const.tile([128, 8, O], dt.bfloat16)       # w0t chunked [c=128g+p]
        nc.sync.dma_start(lw0[:], w0t.ap().rearrange("(g p) o -> p g o", p=128))
        lw1 = const.tile([128, 32, O], dt.bfloat16)
        nc.sync.dma_start(lw1[:], w1t.ap().rearrange("(g p) o -> p g o", p=128))
        lw2 = const.tile([128, 32, O], dt.bfloat16)
        nc.sync.dma_start(lw2[:], w2t.ap().rearrange("(g p) o -> p g o", p=128))

        bias0 = const.tile([128, 2], dt.float32)
        nc.sync.dma_start(bias0[:], b0.ap().rearrange("(m p) -> p m", p=128))
        bias1 = const.tile([128, 2], dt.float32)
        nc.sync.dma_start(bias1[:], b1.ap().rearrange("(m p) -> p m", p=128))
        bias2 = const.tile([128, 2], dt.float32)
        nc.sync.dma_start(bias2[:], b2.ap().rearrange("(m p) -> p m", p=128))

        # pooled accumulators [o_chunk 128, batch 512]
        P0 = const.tile([128, n_tiles * NB], dt.float32)
        P1 = const.tile([128, n_tiles * NB], dt.float32)
        P2a = const.tile([128, n_tiles * NB], dt.float32)
        P2b = const.tile([128, n_tiles * NB], dt.float32)

        # ---- per-tile pools ----
        xr_pool = ctx.enter_context(tc.tile_pool(name="xr", bufs=3))
        xr0_pool = ctx.enter_context(tc.tile_pool(name="xr0", bufs=2))
        x0r_pool = ctx.enter_context(tc.tile_pool(name="x0r", bufs=2))
        z_pool = ctx.enter_context(tc.tile_pool(name="z", bufs=3))
        z0_pool = ctx.enter_context(tc.tile_pool(name="z0", bufs=2))
        h_pool = ctx.enter_context(tc.tile_pool(name="h", bufs=3))
        r_pool = ctx.enter_context(tc.tile_pool(name="r", bufs=4))
        psum_pool = ctx.enter_context(tc.tile_pool(name="ps", bufs=6, space="PSUM"))

        for t in range(n_tiles):
            # xb is [F, BC, E] (host pre-transposed); tile slice: [F, NB, E]
            xsl = xb.ap()[:, t * NB:(t + 1) * NB, :].rearrange(
                "f b e -> f (b e)")  # [32, 512], col-contiguous per f

            # XR halves: [128, 16, 512]; value[p, fh, col] = xsl[f0+fh, col]
            xrh = []
            for half in range(2):
                xr_t = xr_pool.tile([128, 16, COLS], dt.bfloat16,
                                    name=f"xr{half}", tag="xr")
                src = xsl[half * 16:(half + 1) * 16, :] \
                    .unsqueeze(0).broadcast_to([128, 16, COLS])
                nc.sync.dma_start(xr_t[:], src)
                xrh.append(xr_t)

            # XR0: [128, 8, 512]; value[p, g, col] = xsl[4g + (p>>5), col]
            xr0 = xr0_pool.tile([128, 8, COLS], dt.bfloat16)
            for fh in range(4):
                src = xsl.rearrange("(g fh) c -> fh g c", fh=4)[fh] \
                    .unsqueeze(0).broadcast_to([32, 8, COLS])
                nc.sync.dma_start(xr0[fh * 32:(fh + 1) * 32], src)

            # x0rep: [128, 512]; value[p, col] = xsl[p & 31, col]
            x0rep = x0r_pool.tile([128, COLS], dt.bfloat16)
            for k in range(4):
                nc.sync.dma_start(x0rep[k * 32:(k + 1) * 32], xsl)

            # ---- layer 0 ----
            z0 = z0_pool.tile([128, 8, COLS], dt.bfloat16)
            nc.vector.tensor_tensor(
                z0[:], xr0[:],
                x0rep[:].unsqueeze(1).broadcast_to([128, 8, COLS]), ALU.mult)

            ps0 = [psum_pool.tile([128, COLS], dt.float32, name="ps0a", tag="ps"),
                   psum_pool.tile([128, COLS], dt.float32, name="ps0b", tag="ps")]
            for m in range(2):
                for g in range(8):
                    nc.tensor.matmul(
                        ps0[m][:], lw0[:, g, m * 128:(m + 1) * 128], z0[:, g, :],
                        start=(g == 0), stop=(g == 7))

            r0 = r_pool.tile([128, COLS], dt.float32, name="r0", tag="r")
            nc.scalar.activation(r0[:], ps0[0][:], AF.Relu, bias=bias0[:, 0:1])
            h1 = h_pool.tile([128, COLS], dt.bfloat16, name="h1", tag="h")
            nc.scalar.activation(h1[:], ps0[1][:], AF.Relu, bias=bias0[:, 1:2])
            if debug and t == 0:
                dtmp = const.tile([128, 8 * COLS], dt.float32, name="dz0t")
                nc.vector.tensor_copy(dtmp[:], z0[:].rearrange("p a b -> p (a b)"))
                nc.sync.dma_start(dbg["dz0"].ap(), dtmp[:])
                dtmp2 = const.tile([128, COLS], dt.float32, name="dh1t")
                nc.vector.tensor_copy(dtmp2[:], h1[:])
                nc.sync.dma_start(dbg["dh1"].ap(), dtmp2[:])
            nc.vector.tensor_reduce(
                P0[:, t * NB:(t + 1) * NB],
                r0[:].rearrange("p (b e) -> p b e", e=E), AX.X, ALU.add)

            # ---- layers 1, 2 ----
            h_cur = h1
            for layer, (lw, bias, rnames) in enumerate(
                    [(lw1, bias1, ("r1",)), (lw2, bias2, ("r2a", "r2b"))]):
                zh = []
                for half in range(2):
                    z_t = z_pool.tile([128, 16, COLS], dt.bfloat16,
                                      name=f"z{layer}{half}", tag="z")
                    nc.vector.tensor_tensor(
                        z_t[:], xrh[half][:],
                        h_cur[:].unsqueeze(1).broadcast_to([128, 16, COLS]),
                        ALU.mult)
                    zh.append(z_t)
                ps = [psum_pool.tile([128, COLS], dt.float32, name=f"psl{layer}a", tag="ps"),
                      psum_pool.tile([128, COLS], dt.float32, name=f"psl{layer}b", tag="ps")]
                for m in range(2):
                    for half in range(2):
                        for g in range(16):
                            nc.tensor.matmul(
                                ps[m][:], lw[:, half * 16 + g, m * 128:(m + 1) * 128],
                                zh[half][:, g, :],
                                start=(half == 0 and g == 0),
                                stop=(half == 1 and g == 15))
                if layer == 0:
                    r1 = r_pool.tile([128, COLS], dt.float32, name="r1", tag="r")
                    nc.scalar.activation(r1[:], ps[0][:], AF.Relu, bias=bias[:, 0:1])
                    h2 = h_pool.tile([128, COLS], dt.bfloat16, name="h2", tag="h")
                    nc.scalar.activation(h2[:], ps[1][:], AF.Relu, bias=bias[:, 1:2])
                    nc.vector.tensor_reduce(
                        P1[:, t * NB:(t + 1) * NB],
                        r1[:].rearrange("p (b e) -> p b e", e=E), AX.X, ALU.add)
                    if debug and t == 0:
                        dtmp3 = const.tile([128, COLS], dt.float32, name="dh2t")
                        nc.vector.tensor_copy(dtmp3[:], h2[:])
                        nc.sync.dma_start(dbg["dh2"].ap(), dtmp3[:])
                    h_cur = h2
                else:
                    r2a = r_pool.tile([128, COLS], dt.float32, name="r2a", tag="r")
                    nc.scalar.activation(r2a[:], ps[0][:], AF.Relu, bias=bias[:, 0:1])
                    r2b = r_pool.tile([128, COLS], dt.float32, name="r2b", tag="r")
                    nc.scalar.activation(r2b[:], ps[1][:], AF.Relu, bias=bias[:, 1:2])
                    nc.vector.tensor_reduce(
                        P2a[:, t * NB:(t + 1) * NB],
                        r2a[:].rearrange("p (b e) -> p b e", e=E), AX.X, ALU.add)
                    nc.vector.tensor_reduce(
                        P2b[:, t * NB:(t + 1) * NB],
                        r2b[:].rearrange("p (b e) -> p b e", e=E), AX.X, ALU.add)

        # ---- ship pooled accumulators; tiny FC happens on host ----
        for c, P in enumerate([P0, P1, P2a, P2b]):
            nc.sync.dma_start(pout.ap()[c], P[:])
        if debug:
            for nm, P in [("dP0", P0), ("dP1", P1), ("dP2a", P2a), ("dP2b", P2b)]:
                nc.sync.dma_start(dbg[nm].ap(), P[:])

    nc.compile()
    return nc


def _prep_inputs(x, w0, b0, w1, b1, w2, b2, fc_w, fc_b):
    bf16 = ml_dtypes.bfloat16
    xb = np.asarray(x, dtype=np.float32).astype(bf16)
    w0t = np.ascontiguousarray(np.asarray(w0, np.float32).T).astype(bf16)
    w1t = np.ascontiguousarray(np.asarray(w1, np.float32).T).astype(bf16)
    w2t = np.ascontiguousarray(np.asarray(w2, np.float32).T).astype(bf16)
    common = {
        "w0t": w0t, "w1t": w1t, "w2t": w2t,
        "b0": np.ascontiguousarray(np.asarray(b0, np.float32)),
        "b1": np.ascontiguousarray(np.asarray(b1, np.float32)),
        "b2": np.ascontiguousarray(np.asarray(b2, np.float32)),
    }
    in_maps = []
    for c in range(N_CORES):
        m = dict(common)
        m["xb"] = np.ascontiguousarray(
            xb[c * BC:(c + 1) * BC].transpose(1, 0, 2))
        in_maps.append(m)
    return in_maps


def kernel(x, w0, b0, w1, b1, w2, b2, fc_w, fc_b, **kw):
    from concourse.bass_utils import run_bass_kernel_spmd

    if "nc" not in _CACHE:
        _CACHE["nc"] = _build()
    nc = _CACHE["nc"]
    in_maps = _prep_inputs(x, w0, b0, w1, b1, w2, b2, fc_w, fc_b)
    res = run_bass_kernel_spmd(nc, in_maps, list(range(N_CORES)))
    fcw = np.asarray(fc_w, np.float32).reshape(4, 128)
    ys = []
    for c in range(N_CORES):
        p = res.results[c]["pout"]  # [4, 128, BC]
        ys.append(np.einsum('cp,cpb->b', fcw, p.astype(np.float32)))
    out = np.concatenate(ys).reshape(B, 1).astype(np.float32)
    out = out + np.asarray(fc_b, np.float32).reshape(1, 1)
    return out



# revision 2
# speedup vs baseline: 1.8026x; 1.8026x over previous
"""CIN (xDeepFM CompressedInteractionNetwork) forward on 8 TRN2 NeuronCores.

Strategy (pure data parallelism, hardcoded from the problem spec):
  - batch 4096 -> 512 per core; 64 tiles of 8 batch elements; matmul free
    dim = 512 columns = (8 batch x 64 embed).
  - layer l: out[o, col] = relu( sum_c W[o,c] * z[c, col] + b[o] ) where
    z[f*Hin+j, col] = x0[f, col] * h[j, col].  z is materialized as bf16
    tensor_tensor multiplies (x0 rows partition-broadcast via DMA, h
    broadcast along a stride-0 free dim).
  - matmuls in bf16 (full PE rate), fp32 PSUM; ScalarE applies bias+relu
    out of PSUM; DVE pools (sum over embed); final FC on host.

Perf structure (v2):
  - software-pipelined PE stream: per iteration i the tensor engine runs
    L0(i), L2(i-1), L1(i); each layer's h-producing chain (m=1) first.
    All PE dependencies are then ~a full layer old -> no 8us z-build
    stalls (the v1 baseline lost 1.0ms/core to 128 such gaps and sat at
    the 1.2GHz p-state; a dense stream sustains 2.4GHz).
  - z0-build on the Pool engine (gpsimd), z1/z2 builds + pooling reduces
    on DVE (r tiles bf16 for 2x DVE reduce rate).
  - host pre-lays x out per-tile ([tile, f, b, e] and a f=4g+fh variant)
    so every partition-broadcast DMA reads a contiguous 8/16KB block ->
    16KB descriptors instead of 1KB (v1: 16 DMA engines x 1.65ms busy).
"""

import sys

sys.path.insert(0, "/opt/trn_rl_repo")

import numpy as np
import ml_dtypes
from contextlib import ExitStack

N_CORES = 8
B = 4096
F = 32
E = 64
BC = B // N_CORES  # 512 batch elements per core
NB = 8             # batch elements per tile
COLS = NB * E      # 512 matmul columns per tile
NT = BC // NB      # 64 tiles per core
O = 256            # conv out channels per layer

_CACHE = {}


def _build(n_tiles=NT):
    import concourse.bass as bass  # noqa: F401
    import concourse.mybir as mybir
    import concourse.tile as tile
    from concourse import bacc

    dt = mybir.dt
    AF = mybir.ActivationFunctionType
    ALU = mybir.AluOpType
    AX = mybir.AxisListType

    nc = bacc.Bacc("TRN2", target_bir_lowering=False, debug=False,
                   num_devices=N_CORES)

    # x pre-laid out per tile: xtile[t, f, (b e)] and xq[t, fh, (g b e)]
    # (row order f = 4*g + fh) so broadcast DMAs read contiguous blocks.
    xtile = nc.declare_dram_parameter("xtile", [n_tiles, F, COLS],
                                      dt.bfloat16, isOutput=False)
    xq = nc.declare_dram_parameter("xq", [n_tiles, 4, 8 * COLS],
                                   dt.bfloat16, isOutput=False)
    w0t = nc.declare_dram_parameter("w0t", [F * F, O], dt.bfloat16, isOutput=False)
    w1t = nc.declare_dram_parameter("w1t", [F * 128, O], dt.bfloat16, isOutput=False)
    w2t = nc.declare_dram_parameter("w2t", [F * 128, O], dt.bfloat16, isOutput=False)
    b0 = nc.declare_dram_parameter("b0", [O], dt.float32, isOutput=False)
    b1 = nc.declare_dram_parameter("b1", [O], dt.float32, isOutput=False)
    b2 = nc.declare_dram_parameter("b2", [O], dt.float32, isOutput=False)
    pout = nc.declare_dram_parameter("pout", [4, 128, n_tiles * NB],
                                     dt.float32, isOutput=True)

    with ExitStack() as ctx:
        tc = ctx.enter_context(tile.TileContext(nc))
        const = ctx.enter_context(tc.tile_pool(name="const", bufs=1))

        # ---- persistent weights / biases ----
        lw0 = const.tile([128, 8, O], dt.bfloat16)       # w0t chunked [c=128g+p]
        nc.sync.dma_start(lw0[:], w0t.ap().rearrange("(g p) o -> p g o", p=128))
        lw1 = const.tile([128, 32, O], dt.bfloat16)
        nc.sync.dma_start(lw1[:], w1t.ap().rearrange("(g p) o -> p g o", p=128))
        lw2 = const.tile([128, 32, O], dt.bfloat16)
        nc.sync.dma_start(lw2[:], w2t.ap().rearrange("(g p) o -> p g o", p=128))

        bias0 = const.tile([128, 2], dt.float32)
        nc.sync.dma_start(bias0[:], b0.ap().rearrange("(m p) -> p m", p=128))
        bias1 = const.tile([128, 2], dt.float32)
        nc.sync.dma_start(bias1[:], b1.ap().rearrange("(m p) -> p m", p=128))
        bias2 = const.tile([128, 2], dt.float32)
        nc.sync.dma_start(bias2[:], b2.ap().rearrange("(m p) -> p m", p=128))

        # pooled accumulators [o_chunk 128, batch 512]
        P0 = const.tile([128, n_tiles * NB], dt.float32)
        P1 = const.tile([128, n_tiles * NB], dt.float32)
        P2a = const.tile([128, n_tiles * NB], dt.float32)
        P2b = const.tile([128, n_tiles * NB], dt.float32)

        # ---- rotating pools ----
        xr_pool = ctx.enter_context(tc.tile_pool(name="xr", bufs=3))
        xr0_pool = ctx.enter_context(tc.tile_pool(name="xr0", bufs=2))
        x0r_pool = ctx.enter_context(tc.tile_pool(name="x0r", bufs=2))
        z0_pool = ctx.enter_context(tc.tile_pool(name="z0", bufs=2))
        z1_pool = ctx.enter_context(tc.tile_pool(name="z1", bufs=2))
        z2_pool = ctx.enter_context(tc.tile_pool(name="z2", bufs=2))
        h_pool = ctx.enter_context(tc.tile_pool(name="h", bufs=3))
        r_pool = ctx.enter_context(tc.tile_pool(name="r", bufs=4))
        psum_pool = ctx.enter_context(tc.tile_pool(name="ps", bufs=6, space="PSUM"))

        # per-tile state carried across pipeline stages
        xrh = [None] * n_tiles   # [2 x tile [128,16,COLS]]
        z0t = [None] * n_tiles
        z2t = [None] * n_tiles
        x0rep_t = [None] * n_tiles
        xr0_t = [None] * n_tiles

        def emit_dma(t):
            """Prefetch tile t's x data (broadcast replication)."""
            halves = []
            for half in range(2):
                xr_t = xr_pool.tile([128, 16, COLS], dt.bfloat16,
                                    name=f"xr{half}", tag="xr")
                src = xtile.ap()[t, half * 16:(half + 1) * 16, :] \
                    .unsqueeze(0).broadcast_to([128, 16, COLS])
                nc.sync.dma_start(xr_t[:], src)
                halves.append(xr_t)
            xrh[t] = halves

            xr0 = xr0_pool.tile([128, 8, COLS], dt.bfloat16)
            for fh in range(4):
                src = xq.ap()[t, fh].rearrange("(g c) -> g c", c=COLS) \
                    .unsqueeze(0).broadcast_to([32, 8, COLS])
                nc.sync.dma_start(xr0[fh * 32:(fh + 1) * 32], src)
            xr0_t[t] = xr0

            x0rep = x0r_pool.tile([128, COLS], dt.bfloat16)
            for k in range(4):
                nc.sync.dma_start(x0rep[k * 32:(k + 1) * 32], xtile.ap()[t])
            x0rep_t[t] = x0rep

        def emit_z0(t):
            """z0(t) on the Pool engine (keeps DVE free for z1/z2)."""
            z0 = z0_pool.tile([128, 8, COLS], dt.bfloat16)
            nc.gpsimd.tensor_tensor(
                z0[:], xr0_t[t][:],
                x0rep_t[t][:].unsqueeze(1).broadcast_to([128, 8, COLS]),
                ALU.mult)
            z0t[t] = z0

        def reduce_into(P, t, r_t):
            nc.vector.tensor_reduce(
                P[:, t * NB:(t + 1) * NB],
                r_t[:].rearrange("p (b e) -> p b e", e=E), AX.X, ALU.add)

        # ---- preamble: tile 0 inputs + z0(0) ----
        emit_dma(0)
        emit_z0(0)

        for i in range(n_tiles + 1):
            if i + 1 < n_tiles:
                emit_dma(i + 1)

            if i < n_tiles:
                # -- PE: L0(i); m=1 (h-half) first --
                ps0 = {m: psum_pool.tile([128, COLS], dt.float32,
                                         name=f"ps0{m}", tag="ps")
                       for m in (1, 0)}
                for m in (1, 0):
                    for g in range(8):
                        nc.tensor.matmul(
                            ps0[m][:], lw0[:, g, m * 128:(m + 1) * 128],
                            z0t[i][:, g, :], start=(g == 0), stop=(g == 7))
                h1 = h_pool.tile([128, COLS], dt.bfloat16, name="h1", tag="h")
                nc.scalar.activation(h1[:], ps0[1][:], AF.Relu, bias=bias0[:, 1:2])
                r0 = r_pool.tile([128, COLS], dt.bfloat16, name="r0", tag="r")
                nc.scalar.activation(r0[:], ps0[0][:], AF.Relu, bias=bias0[:, 0:1])

                # -- DVE: z1(i) halves --
                z1h = []
                for half in range(2):
                    z_t = z1_pool.tile([128, 16, COLS], dt.bfloat16,
                                       name=f"z1{half}", tag="z1")
                    nc.vector.tensor_tensor(
                        z_t[:], xrh[i][half][:],
                        h1[:].unsqueeze(1).broadcast_to([128, 16, COLS]),
                        ALU.mult)
                    z1h.append(z_t)

            if i >= 1:
                # -- PE: L2(i-1) --
                c = i - 1
                ps2 = {m: psum_pool.tile([128, COLS], dt.float32,
                                         name=f"ps2{m}", tag="ps")
                       for m in (0, 1)}
                for m in (0, 1):
                    for half in range(2):
                        for g in range(16):
                            nc.tensor.matmul(
                                ps2[m][:],
                                lw2[:, half * 16 + g, m * 128:(m + 1) * 128],
                                z2t[c][half][:, g, :],
                                start=(half == 0 and g == 0),
                                stop=(half == 1 and g == 15))
                r2a = r_pool.tile([128, COLS], dt.bfloat16, name="r2a", tag="r")
                nc.scalar.activation(r2a[:], ps2[0][:], AF.Relu, bias=bias2[:, 0:1])
                r2b = r_pool.tile([128, COLS], dt.bfloat16, name="r2b", tag="r")
                nc.scalar.activation(r2b[:], ps2[1][:], AF.Relu, bias=bias2[:, 1:2])

            # -- Pool: z0(i+1) (deps: tile i+1 DMAs only) --
            if i + 1 < n_tiles:
                emit_z0(i + 1)

            if i < n_tiles:
                # -- PE: L1(i); m=1 (h-half) first --
                ps1 = {m: psum_pool.tile([128, COLS], dt.float32,
                                         name=f"ps1{m}", tag="ps")
                       for m in (1, 0)}
                for m in (1, 0):
                    for half in range(2):
                        for g in range(16):
                            nc.tensor.matmul(
                                ps1[m][:],
                                lw1[:, half * 16 + g, m * 128:(m + 1) * 128],
                                z1h[half][:, g, :],
                                start=(half == 0 and g == 0),
                                stop=(half == 1 and g == 15))
                h2 = h_pool.tile([128, COLS], dt.bfloat16, name="h2", tag="h")
                nc.scalar.activation(h2[:], ps1[1][:], AF.Relu, bias=bias1[:, 1:2])
                r1 = r_pool.tile([128, COLS], dt.bfloat16, name="r1", tag="r")
                nc.scalar.activation(r1[:], ps1[0][:], AF.Relu, bias=bias1[:, 0:1])

                # -- DVE: z2(i) halves --
                z2h = []
                for half in range(2):
                    z_t = z2_pool.tile([128, 16, COLS], dt.bfloat16,
                                       name=f"z2{half}", tag="z2")
                    nc.vector.tensor_tensor(
                        z_t[:], xrh[i][half][:],
                        h2[:].unsqueeze(1).broadcast_to([128, 16, COLS]),
                        ALU.mult)
                    z2h.append(z_t)
                z2t[i] = z2h

                # -- DVE: pooling reduces --
                reduce_into(P0, i, r0)
                if i >= 1:
                    reduce_into(P2a, i - 1, r2a)
                    reduce_into(P2b, i - 1, r2b)
                reduce_into(P1, i, r1)
            else:
                reduce_into(P2a, i - 1, r2a)
                reduce_into(P2b, i - 1, r2b)

        # ---- ship pooled accumulators; tiny FC happens on host ----
        for c, P in enumerate([P0, P1, P2a, P2b]):
            nc.sync.dma_start(pout.ap()[c], P[:])

    nc.compile()
    return nc


def _prep_inputs(x, w0, b0, w1, b1, w2, b2, fc_w, fc_b):
    bf16 = ml_dtypes.bfloat16
    xb = np.asarray(x, dtype=np.float32).astype(bf16)
    w0t = np.ascontiguousarray(np.asarray(w0, np.float32).T).astype(bf16)
    w1t = np.ascontiguousarray(np.asarray(w1, np.float32).T).astype(bf16)
    w2t = np.ascontiguousarray(np.asarray(w2, np.float32).T).astype(bf16)
    common = {
        "w0t": w0t, "w1t": w1t, "w2t": w2t,
        "b0": np.ascontiguousarray(np.asarray(b0, np.float32)),
        "b1": np.ascontiguousarray(np.asarray(b1, np.float32)),
        "b2": np.ascontiguousarray(np.asarray(b2, np.float32)),
    }
    in_maps = []
    for c in range(N_CORES):
        m = dict(common)
        xc = xb[c * BC:(c + 1) * BC]                     # [BC, F, E]
        # xtile[t, f, (b e)] = x[8t+b, f, e]
        xt = np.ascontiguousarray(
            xc.reshape(NT, NB, F, E).transpose(0, 2, 1, 3).reshape(NT, F, COLS))
        m["xtile"] = xt
        # xq[t, fh, (g b e)]: row order f = 4g + fh
        m["xq"] = np.ascontiguousarray(
            xt.reshape(NT, 8, 4, COLS).transpose(0, 2, 1, 3).reshape(NT, 4, 8 * COLS))
        in_maps.append(m)
    return in_maps


def kernel(x, w0, b0, w1, b1, w2, b2, fc_w, fc_b, **kw):
    from concourse.bass_utils import run_bass_kernel_spmd

    if "nc" not in _CACHE:
        _CACHE["nc"] = _build()
    nc = _CACHE["nc"]
    in_maps = _prep_inputs(x, w0, b0, w1, b1, w2, b2, fc_w, fc_b)
    res = run_bass_kernel_spmd(nc, in_maps, list(range(N_CORES)))
    fcw = np.asarray(fc_w, np.float32).reshape(4, 128)
    ys = []
    for c in range(N_CORES):
        p = res.results[c]["pout"]  # [4, 128, BC]
        ys.append(np.einsum('cp,cpb->b', fcw, p.astype(np.float32)))
    out = np.concatenate(ys).reshape(B, 1).astype(np.float32)
    out = out + np.asarray(fc_b, np.float32).reshape(1, 1)
    return out


# revision 6
# speedup vs baseline: 1.8275x; 1.0138x over previous
"""CIN (xDeepFM CompressedInteractionNetwork) forward on 8 TRN2 NeuronCores.

Strategy (pure data parallelism, hardcoded from the problem spec):
  - batch 4096 -> 512 per core; 64 tiles of 8 batch elements; matmul free
    dim = 512 columns = (8 batch x 64 embed).
  - layer l: out[o, col] = relu( sum_c W[o,c] * z[c, col] + b[o] ) where
    z[f*Hin+j, col] = x0[f, col] * h[j, col].  z is materialized as bf16
    tensor_tensor multiplies (x0 rows partition-broadcast via DMA, h
    broadcast along a stride-0 free dim).
  - matmuls in bf16 (full PE rate), fp32 PSUM; ScalarE applies bias+relu
    out of PSUM; DVE pools (sum over embed); final FC on host.

Perf structure (v2):
  - software-pipelined PE stream: per iteration i the tensor engine runs
    L0(i), L2(i-1), L1(i); each layer's h-producing chain (m=1) first.
    All PE dependencies are then ~a full layer old -> no 8us z-build
    stalls (the v1 baseline lost 1.0ms/core to 128 such gaps and sat at
    the 1.2GHz p-state; a dense stream sustains 2.4GHz).
  - z0-build on the Pool engine (gpsimd), z1/z2 builds + pooling reduces
    on DVE (r tiles bf16 for 2x DVE reduce rate).
  - host pre-lays x out per-tile ([tile, f, b, e] and a f=4g+fh variant)
    so every partition-broadcast DMA reads a contiguous 8/16KB block ->
    16KB descriptors instead of 1KB (v1: 16 DMA engines x 1.65ms busy).
"""

import sys

sys.path.insert(0, "/opt/trn_rl_repo")

import numpy as np
import ml_dtypes
from contextlib import ExitStack

N_CORES = 8
B = 4096
F = 32
E = 64
BC = B // N_CORES  # 512 batch elements per core
NB = 8             # batch elements per tile
COLS = NB * E      # 512 matmul columns per tile
NT = BC // NB      # 64 tiles per core
O = 256            # conv out channels per layer

_CACHE = {}


def _build(n_tiles=NT):
    import concourse.bass as bass  # noqa: F401
    import concourse.mybir as mybir
    import concourse.tile as tile
    from concourse import bacc

    dt = mybir.dt
    AF = mybir.ActivationFunctionType
    ALU = mybir.AluOpType
    AX = mybir.AxisListType

    nc = bacc.Bacc("TRN2", target_bir_lowering=False, debug=False,
                   num_devices=N_CORES)

    # x pre-laid out per tile: xtile[t, f, (b e)] and xq[t, fh, (g b e)]
    # (row order f = 4*g + fh) so broadcast DMAs read contiguous blocks.
    xtile = nc.declare_dram_parameter("xtile", [n_tiles, F, COLS],
                                      dt.bfloat16, isOutput=False)
    xq = nc.declare_dram_parameter("xq", [n_tiles, 4, 8 * COLS],
                                   dt.bfloat16, isOutput=False)
    # weights host-prepped to [p, g, o] (per-partition contiguous blocks)
    w0t = nc.declare_dram_parameter("w0t", [128, 8 * O], dt.bfloat16, isOutput=False)
    w1t = nc.declare_dram_parameter("w1t", [128, 32 * O], dt.bfloat16, isOutput=False)
    w2t = nc.declare_dram_parameter("w2t", [128, 32 * O], dt.bfloat16, isOutput=False)
    b0 = nc.declare_dram_parameter("b0", [O], dt.float32, isOutput=False)
    b1 = nc.declare_dram_parameter("b1", [O], dt.float32, isOutput=False)
    b2 = nc.declare_dram_parameter("b2", [O], dt.float32, isOutput=False)
    pout = nc.declare_dram_parameter("pout", [4, 128, n_tiles * NB],
                                     dt.float32, isOutput=True)

    with ExitStack() as ctx:
        tc = ctx.enter_context(tile.TileContext(nc))
        const = ctx.enter_context(tc.tile_pool(name="const", bufs=1))

        # ---- persistent weights / biases (DMAs interleaved below so tile-0
        #      prefetch isn't stuck behind 4.5MB of weights) ----
        lw0 = const.tile([128, 8, O], dt.bfloat16)       # w chunk [c=128g+p]
        lw1 = const.tile([128, 32, O], dt.bfloat16)
        lw2 = const.tile([128, 32, O], dt.bfloat16)
        bias0 = const.tile([128, 2], dt.float32)
        bias1 = const.tile([128, 2], dt.float32)
        bias2 = const.tile([128, 2], dt.float32)

        # pooled accumulators [o_chunk 128, batch 512]
        P0 = const.tile([128, n_tiles * NB], dt.float32)
        P1 = const.tile([128, n_tiles * NB], dt.float32)
        P2a = const.tile([128, n_tiles * NB], dt.float32)
        P2b = const.tile([128, n_tiles * NB], dt.float32)

        # ---- rotating pools ----
        xr_pool = ctx.enter_context(tc.tile_pool(name="xr", bufs=3))
        xr0_pool = ctx.enter_context(tc.tile_pool(name="xr0", bufs=2))
        x0r_pool = ctx.enter_context(tc.tile_pool(name="x0r", bufs=2))
        z0_pool = ctx.enter_context(tc.tile_pool(name="z0", bufs=2))
        z1_pool = ctx.enter_context(tc.tile_pool(name="z1", bufs=2))
        z2_pool = ctx.enter_context(tc.tile_pool(name="z2", bufs=2))
        h_pool = ctx.enter_context(tc.tile_pool(name="h", bufs=3))
        r_pool = ctx.enter_context(tc.tile_pool(name="r", bufs=4))
        psum_pool = ctx.enter_context(tc.tile_pool(name="ps", bufs=6, space="PSUM"))

        # per-tile state carried across pipeline stages
        xrh = [None] * n_tiles   # [2 x tile [128,16,COLS]]
        z0t = [None] * n_tiles
        z2t = [None] * n_tiles
        x0rep_t = [None] * n_tiles
        xr0_t = [None] * n_tiles

        def emit_dma(t):
            """Prefetch tile t's x data (broadcast replication).
            xr0/x0rep first: they gate z0(t) which gates L0(t)."""
            xr0 = xr0_pool.tile([128, 8, COLS], dt.bfloat16)
            for fh in range(4):
                src = xq.ap()[t, fh].rearrange("(g c) -> g c", c=COLS) \
                    .unsqueeze(0).broadcast_to([32, 8, COLS])
                nc.sync.dma_start(xr0[fh * 32:(fh + 1) * 32], src)
            xr0_t[t] = xr0

            x0rep = x0r_pool.tile([128, COLS], dt.bfloat16)
            for k in range(4):
                nc.sync.dma_start(x0rep[k * 32:(k + 1) * 32], xtile.ap()[t])
            x0rep_t[t] = x0rep

            halves = []
            for half in range(2):
                xr_t = xr_pool.tile([128, 16, COLS], dt.bfloat16,
                                    name=f"xr{half}", tag="xr")
                src = xtile.ap()[t, half * 16:(half + 1) * 16, :] \
                    .unsqueeze(0).broadcast_to([128, 16, COLS])
                nc.sync.dma_start(xr_t[:], src)
                halves.append(xr_t)
            xrh[t] = halves

        def emit_z0(t, eng=None):
            """z0(t) on the Pool engine (keeps DVE free for z1/z2)."""
            z0 = z0_pool.tile([128, 8, COLS], dt.bfloat16)
            (eng or nc.gpsimd).tensor_tensor(
                z0[:], xr0_t[t][:],
                x0rep_t[t][:].unsqueeze(1).broadcast_to([128, 8, COLS]),
                ALU.mult)
            z0t[t] = z0

        def reduce_into(P, t, r_t):
            nc.vector.tensor_reduce(
                P[:, t * NB:(t + 1) * NB],
                r_t[:].rearrange("p (b e) -> p b e", e=E), AX.X, ALU.add)

        # ---- preamble: lw0 + tile 0 inputs + z0(0), then the big weights ----
        nc.sync.dma_start(lw0[:], w0t.ap().rearrange("p (g o) -> p g o", o=O))
        nc.sync.dma_start(bias0[:], b0.ap().rearrange("(m p) -> p m", p=128))
        emit_dma(0)
        emit_z0(0, eng=nc.vector)   # DVE: 2.3us vs 12us on Pool; gates 1st matmul
        nc.sync.dma_start(lw1[:], w1t.ap().rearrange("p (g o) -> p g o", o=O))
        nc.sync.dma_start(bias1[:], b1.ap().rearrange("(m p) -> p m", p=128))
        nc.sync.dma_start(lw2[:], w2t.ap().rearrange("p (g o) -> p g o", o=O))
        nc.sync.dma_start(bias2[:], b2.ap().rearrange("(m p) -> p m", p=128))

        for i in range(n_tiles + 1):
            if i + 1 < n_tiles:
                emit_dma(i + 1)

            if i < n_tiles:
                # -- PE: L0(i); m=1 (h-half) first --
                ps0 = {m: psum_pool.tile([128, COLS], dt.float32,
                                         name=f"ps0{m}", tag="ps")
                       for m in (1, 0)}
                for m in (1, 0):
                    for g in range(8):
                        nc.tensor.matmul(
                            ps0[m][:], lw0[:, g, m * 128:(m + 1) * 128],
                            z0t[i][:, g, :], start=(g == 0), stop=(g == 7))
                h1 = h_pool.tile([128, COLS], dt.bfloat16, name="h1", tag="h")
                nc.scalar.activation(h1[:], ps0[1][:], AF.Relu, bias=bias0[:, 1:2])
                r0 = r_pool.tile([128, COLS], dt.bfloat16, name="r0", tag="r")
                nc.scalar.activation(r0[:], ps0[0][:], AF.Relu, bias=bias0[:, 0:1])

                # -- DVE: z1(i) halves --
                z1h = []
                for half in range(2):
                    z_t = z1_pool.tile([128, 16, COLS], dt.bfloat16,
                                       name=f"z1{half}", tag="z1")
                    nc.vector.tensor_tensor(
                        z_t[:], xrh[i][half][:],
                        h1[:].unsqueeze(1).broadcast_to([128, 16, COLS]),
                        ALU.mult)
                    z1h.append(z_t)

            if i >= 1:
                # -- PE: L2(i-1) --
                c = i - 1
                ps2 = {m: psum_pool.tile([128, COLS], dt.float32,
                                         name=f"ps2{m}", tag="ps")
                       for m in (0, 1)}
                for m in (0, 1):
                    for half in range(2):
                        for g in range(16):
                            nc.tensor.matmul(
                                ps2[m][:],
                                lw2[:, half * 16 + g, m * 128:(m + 1) * 128],
                                z2t[c][half][:, g, :],
                                start=(half == 0 and g == 0),
                                stop=(half == 1 and g == 15))
                r2a = r_pool.tile([128, COLS], dt.bfloat16, name="r2a", tag="r")
                nc.scalar.activation(r2a[:], ps2[0][:], AF.Relu, bias=bias2[:, 0:1])
                r2b = r_pool.tile([128, COLS], dt.bfloat16, name="r2b", tag="r")
                nc.scalar.activation(r2b[:], ps2[1][:], AF.Relu, bias=bias2[:, 1:2])

            # -- Pool: z0(i+1) (deps: tile i+1 DMAs only) --
            if i + 1 < n_tiles:
                emit_z0(i + 1)

            if i < n_tiles:
                # -- PE: L1(i); m=1 (h-half) first --
                ps1 = {m: psum_pool.tile([128, COLS], dt.float32,
                                         name=f"ps1{m}", tag="ps")
                       for m in (1, 0)}
                for m in (1, 0):
                    for half in range(2):
                        for g in range(16):
                            nc.tensor.matmul(
                                ps1[m][:],
                                lw1[:, half * 16 + g, m * 128:(m + 1) * 128],
                                z1h[half][:, g, :],
                                start=(half == 0 and g == 0),
                                stop=(half == 1 and g == 15))
                h2 = h_pool.tile([128, COLS], dt.bfloat16, name="h2", tag="h")
                nc.scalar.activation(h2[:], ps1[1][:], AF.Relu, bias=bias1[:, 1:2])
                r1 = r_pool.tile([128, COLS], dt.bfloat16, name="r1", tag="r")
                nc.scalar.activation(r1[:], ps1[0][:], AF.Relu, bias=bias1[:, 0:1])

                # -- DVE: z2(i) halves --
                z2h = []
                for half in range(2):
                    z_t = z2_pool.tile([128, 16, COLS], dt.bfloat16,
                                       name=f"z2{half}", tag="z2")
                    nc.vector.tensor_tensor(
                        z_t[:], xrh[i][half][:],
                        h2[:].unsqueeze(1).broadcast_to([128, 16, COLS]),
                        ALU.mult)
                    z2h.append(z_t)
                z2t[i] = z2h

                # -- DVE: pooling reduces --
                reduce_into(P0, i, r0)
                if i >= 1:
                    reduce_into(P2a, i - 1, r2a)
                    reduce_into(P2b, i - 1, r2b)
                reduce_into(P1, i, r1)
            else:
                reduce_into(P2a, i - 1, r2a)
                reduce_into(P2b, i - 1, r2b)

        # ---- ship pooled accumulators; tiny FC happens on host ----
        for c, P in enumerate([P0, P1, P2a, P2b]):
            nc.sync.dma_start(pout.ap()[c], P[:])

    nc.compile()
    return nc


def _prep_inputs(x, w0, b0, w1, b1, w2, b2, fc_w, fc_b):
    bf16 = ml_dtypes.bfloat16
    xb = np.asarray(x, dtype=np.float32).astype(bf16)

    def wprep(w, G):
        # w [O, c] with c = 128*g + p  ->  [p, (g, o)] contiguous per p
        wt = np.asarray(w, np.float32).T.reshape(G, 128, O)  # [g, p, o]
        return np.ascontiguousarray(
            wt.transpose(1, 0, 2).reshape(128, G * O)).astype(bf16)

    w0t = wprep(w0, 8)
    w1t = wprep(w1, 32)
    w2t = wprep(w2, 32)
    common = {
        "w0t": w0t, "w1t": w1t, "w2t": w2t,
        "b0": np.ascontiguousarray(np.asarray(b0, np.float32)),
        "b1": np.ascontiguousarray(np.asarray(b1, np.float32)),
        "b2": np.ascontiguousarray(np.asarray(b2, np.float32)),
    }
    in_maps = []
    for c in range(N_CORES):
        m = dict(common)
        xc = xb[c * BC:(c + 1) * BC]                     # [BC, F, E]
        # xtile[t, f, (b e)] = x[8t+b, f, e]
        xt = np.ascontiguousarray(
            xc.reshape(NT, NB, F, E).transpose(0, 2, 1, 3).reshape(NT, F, COLS))
        m["xtile"] = xt
        # xq[t, fh, (g b e)]: row order f = 4g + fh
        m["xq"] = np.ascontiguousarray(
            xt.reshape(NT, 8, 4, COLS).transpose(0, 2, 1, 3).reshape(NT, 4, 8 * COLS))
        in_maps.append(m)
    return in_maps


def kernel(x, w0, b0, w1, b1, w2, b2, fc_w, fc_b, **kw):
    from concourse.bass_utils import run_bass_kernel_spmd

    if "nc" not in _CACHE:
        _CACHE["nc"] = _build()
    nc = _CACHE["nc"]
    in_maps = _prep_inputs(x, w0, b0, w1, b1, w2, b2, fc_w, fc_b)
    res = run_bass_kernel_spmd(nc, in_maps, list(range(N_CORES)))
    fcw = np.asarray(fc_w, np.float32).reshape(4, 128)
    ys = []
    for c in range(N_CORES):
        p = res.results[c]["pout"]  # [4, 128, BC]
        ys.append(np.einsum('cp,cpb->b', fcw, p.astype(np.float32)))
    out = np.concatenate(ys).reshape(B, 1).astype(np.float32)
    out = out + np.asarray(fc_b, np.float32).reshape(1, 1)
    return out
